# revision 1
# baseline (speedup 1.0000x reference)
"""DirectedGraphConvolution Trainium2 kernel.

Per batch element b (one per NeuronCore, 8 total, data-parallel):
    N_e = H @ W                          [n, dout]
    T1  = G  @ N_e                       [n, dout]
    T2  = G.T @ N_e                      [n, dout]
    rs  = G.sum(-1); cs = G.sum(-2)
    out = [ relu(0.5*(T1 + T2)),             # G_F @ N_e,  G_F = (G+G.T)/2
            relu(G.T @ (T1 / rs[:,None])),   # G_Sin @ N_e
            relu(G  @ (T2 / cs[:,None])) ]   # G_So  @ N_e
(The reference's [n,n] Gram matrices are never materialized - associativity.)

Layouts: matmul computes out[m,n] = sum_p lhsT[p,m]*rhs[p,n].  G is kept
SBUF-resident in natural layout (row index on partitions), which directly
serves the G.T-left products (passes A and C).  G-left products (pass B)
get their stationary GT blocks from on-the-fly PE transposes, software-
pipelined twelve steps ahead of the consuming matmuls (batching the
transpose_mode instructions cuts PE mode-transition overhead).  cs falls
out of a ones-column fused into pass A's moving operand; rs is reduced
on DVE while G streams in.  Pass A: sweep 1 (jt 0-7) is it-outer across
8 PSUM banks so its matmuls track the G DMA arrival; sweep 2 (jt 8-15)
is jt-outer (G resident by then), which accumulates bank-local and
releases banks progressively so pass B's pipeline starts early.  All
matmuls run in float32r (fp32 storage, ~1 cyc/row at even N>=256).
"""

import numpy as np
import concourse.bass as bass
import concourse.mybir as mybir
import concourse.tile as tile
from concourse import bacc
from concourse.bass_utils import run_bass_kernel_spmd
from concourse.masks import make_identity

F32 = mybir.dt.float32
F32R = mybir.dt.float32r
RELU = mybir.ActivationFunctionType.Relu
AX = mybir.AxisListType.X

P = 128
B = 8
N = 2048
NO = N // P            # 16 row tiles
DIN = 256
DOUT = 256
KO = DIN // P          # 2 k tiles for H @ W
W3 = 3 * DOUT
RB = 2 + DOUT + DOUT   # rhs_b columns: [ones ones | N_e | T2'] (f32r needs even widths)


def build():
    nc = bacc.Bacc("TRN2", target_bir_lowering=False)
    G = nc.declare_dram_parameter("G", [N, N], F32, isOutput=False)
    H = nc.declare_dram_parameter("H", [N, DIN], F32, isOutput=False)
    W = nc.declare_dram_parameter("W", [DIN, DOUT], F32, isOutput=False)
    out = nc.declare_dram_parameter("out", [N, W3], F32, isOutput=True)

    G_r = G.rearrange("(o p) j -> p o j", p=P).bitcast(F32R)
    H_r = H.rearrange("(o p) d -> p o d", p=P).bitcast(F32R)
    W_r = W.rearrange("(o p) d -> p o d", p=P).bitcast(F32R)
    out_r = out.rearrange("(o p) d -> p o d", p=P)

    with tile.TileContext(nc) as tc:
        with (
            tc.tile_pool(name="const", bufs=1) as const,
            tc.tile_pool(name="gpool", bufs=1) as gpool,
            tc.tile_pool(name="big", bufs=1) as big,
            tc.tile_pool(name="hin", bufs=3) as hin,
            tc.tile_pool(name="stage", bufs=4) as stage,
            tc.tile_pool(name="gtp", bufs=12) as gtp,
            tc.tile_pool(name="tmpp", bufs=2) as tmpp,
        ):
            # G DMAs own the Sync HWDGE queue exclusively; everything else
            # (W, H, outputs) issues elsewhere so a slot-release wait can
            # never block the G stream behind it.
            g_tiles = [
                gpool.tile([P, N], F32R, tag=f"g{o}", name=f"g{o}")
                for o in range(NO)
            ]
            for o in range(NO):
                nc.sync.dma_start(g_tiles[o][:, 0:N // 2], G_r[:, o, 0:N // 2])
                nc.sync.dma_start(g_tiles[o][:, N // 2:N], G_r[:, o, N // 2:N])

            w_sb = const.tile([P, KO, DOUT], F32R)
            nc.scalar.dma_start(w_sb, W_r)

            ident_f32 = const.tile([P, P], F32)
            make_identity(nc, ident_f32)
            ident = const.tile([P, P], F32R)
            nc.vector.tensor_copy(ident, ident_f32)
            # rhsb[o] columns: [N_e | T2']
            rhsb = [
                big.tile([P, RB], F32R, tag=f"rb{o}", name=f"rb{o}") for o in range(NO)
            ]
            t1 = [
                big.tile([P, DOUT], F32R, tag=f"t1{o}", name=f"t1{o}")
                for o in range(NO)
            ]
            rsinv = const.tile([P, NO, 1], F32)
            ones_f32 = const.tile([P, 1], F32)
            nc.vector.memset(ones_f32, 1.0)
            cs_sb = const.tile([P, NO, 1], F32)
            csinv = const.tile([P, NO, 1], F32)
            for o in range(NO):
                nc.vector.tensor_copy(rhsb[o][:, 0:1], ones_f32)
                nc.vector.tensor_copy(rhsb[o][:, 1:2], ones_f32)

            # ---- N_e = H @ W  (transpose H blocks on PE, then matmul) ----
            with (
                tc.tile_pool(name="ps_ht", bufs=3, space="PSUM") as ps_ht,
                tc.tile_pool(name="ps_ne", bufs=2, space="PSUM") as ps_ne,
            ):
                # H tiles park in rhsb's T2' region (unused until pass A's
                # epilogue) so every H DMA issues immediately with no SBUF
                # slot-release wait -- a waiting DMA would block the shared
                # HWDGE semaphore slots the G stream cycles through.
                # software pipeline: transposes for tile t run while tile
                # t-1's matmuls consume the previous transposed block, so the
                # PE never stalls on the PSUM->SBUF copy between them
                for t in range(NO):
                    nc.scalar.dma_start(rhsb[t][:, 2 + DOUT:RB], H_r[:, t, :])
                hts = {}
                for t in range(NO + 1):
                    if t < NO:
                        h_t = rhsb[t][:, 2 + DOUT:RB]
                        ht_t = hin.tile([P, KO, P], F32R, tag="ht")
                        for kt in range(KO):
                            pt = ps_ht.tile([P, P], F32, tag="pht")
                            nc.tensor.transpose(
                                pt.bitcast(F32R), h_t[:, kt * P:(kt + 1) * P], ident
                            )
                            nc.vector.tensor_copy(ht_t[:, kt, :], pt.bitcast(F32R))
                        hts[t] = ht_t
                    if t >= 1:
                        u = t - 1
                        ht_u = hts.pop(u)
                        pne = ps_ne.tile([P, DOUT], F32, tag="pne")
                        for kt in range(KO):
                            nc.tensor.matmul(
                                pne,
                                ht_u[:, kt, :],
                                w_sb[:, kt, :],
                                start=(kt == 0),
                                stop=(kt == KO - 1),
                            )
                        nc.vector.tensor_copy(rhsb[u][:, 2:2 + DOUT], pne)

                # rs = row sums (DVE) as G tiles land
                for o in range(NO):
                    rs_t = tmpp.tile([P, 1], F32, tag="rs")
                    nc.vector.reduce_sum(rs_t, g_tiles[o].bitcast(F32), axis=AX)
                    nc.vector.reciprocal(rsinv[:, o, :], rs_t)

            # ---- pass A: [cs cs | T2] = G.T @ [ones ones | N_e] ----
            with tc.tile_pool(name="psA", bufs=8, space="PSUM") as psA:
                def a_epilogue(jt, pa):
                    nc.vector.tensor_copy(cs_sb[:, jt, :], pa[:, 0:1])
                    nc.vector.reciprocal(csinv[:, jt, :], pa[:, 0:1])
                    # T2' = T2 / cs  -> rhsb cols [2+DOUT : RB]
                    nc.vector.tensor_scalar_mul(
                        rhsb[jt][:, 2 + DOUT:RB],
                        pa[:, 2:2 + DOUT],
                        csinv[:, jt, 0:1],
                    )

                # sweep 1 (jt 0-7): it-outer across 8 banks, tracks G arrival
                pas = {
                    jt: psA.tile([P, 2 + DOUT], F32, tag="pa", name=f"pa{jt}")
                    for jt in range(8)
                }
                for it in range(NO):
                    for jt in range(8):
                        nc.tensor.matmul(
                            pas[jt],
                            g_tiles[it][:, jt * P:(jt + 1) * P],
                            rhsb[it][:, 0:2 + DOUT],
                            start=(it == 0),
                            stop=(it == NO - 1),
                        )
                for jt in range(8):
                    a_epilogue(jt, pas[jt])

                # sweep 2 (jt 8-15): G is resident by now, so go jt-outer --
                # consecutive matmuls accumulate into one bank (no per-matmul
                # bank cycling) and banks release progressively, letting pass
                # B's transpose pipeline claim PSUM early
                for jt in range(8, NO):
                    pa2 = psA.tile([P, 2 + DOUT], F32, tag="pa", name=f"pa{jt}")
                    for it in range(NO):
                        nc.tensor.matmul(
                            pa2,
                            g_tiles[it][:, jt * P:(jt + 1) * P],
                            rhsb[it][:, 0:2 + DOUT],
                            start=(it == 0),
                            stop=(it == NO - 1),
                        )
                    a_epilogue(jt, pa2)

            # ---- pass B: [T1 | out3raw] = G @ [N_e | T2'] ----
            # stationary GT blocks from PE transposes, pipelined ahead;
            # PSUM->SBUF block copies alternate DVE / ACT
            with (
                tc.tile_pool(name="psB", bufs=3, space="PSUM") as psB,
                tc.tile_pool(name="psT", bufs=5, space="PSUM") as psT,
            ):
                for it in range(NO):
                    pb = psB.tile([P, 2 * DOUT], F32, tag="pb")
                    gts = {}
                    LOOKAHEAD = 12
                    for step in range(NO + LOOKAHEAD):
                        if step < NO:
                            jt = step
                            pt = psT.tile([P, P], F32, tag="ptr")
                            nc.tensor.transpose(
                                pt.bitcast(F32R),
                                g_tiles[it][:, jt * P:(jt + 1) * P],
                                ident,
                            )
                            gt_t = gtp.tile([P, P], F32R, tag="gt")
                            if jt % 2 == 0:
                                nc.vector.tensor_copy(gt_t, pt.bitcast(F32R))
                            else:
                                nc.scalar.copy(gt_t, pt.bitcast(F32R))
                            gts[jt] = gt_t
                        if step >= LOOKAHEAD:
                            jt = step - LOOKAHEAD
                            nc.tensor.matmul(
                                pb,
                                gts.pop(jt),
                                rhsb[jt][:, 2:RB],
                                start=(jt == 0),
                                stop=(jt == NO - 1),
                            )
                    # out1 = relu(0.5*(T1 + cs*T2'))
                    t2r = tmpp.tile([P, DOUT], F32, tag="t2r")
                    nc.vector.tensor_scalar_mul(
                        t2r, rhsb[it][:, 2 + DOUT:RB].bitcast(F32), cs_sb[:, it, 0:1]
                    )
                    nc.vector.tensor_add(t2r, t2r, pb[:, 0:DOUT])
                    o1 = stage.tile([P, DOUT], F32, tag="o1")
                    nc.scalar.activation(o1, t2r, RELU, scale=0.5)
                    nc.sync.dma_start(out_r[:, it, 0:DOUT], o1)
                    # T1' = T1 / rs
                    nc.vector.tensor_scalar_mul(
                        t1[it], pb[:, 0:DOUT], rsinv[:, it, 0:1]
                    )
                    # out3 = relu(G @ T2')
                    o3 = stage.tile([P, DOUT], F32, tag="o3")
                    nc.scalar.activation(o3, pb[:, DOUT:2 * DOUT], RELU)
                    nc.sync.dma_start(out_r[:, it, 2 * DOUT:W3], o3)

            # ---- pass C: out2 = relu(G.T @ T1') ----
            with tc.tile_pool(name="psC", bufs=6, space="PSUM") as psC:
                for jt in range(NO):
                    pc = psC.tile([P, DOUT], F32, tag="pc")
                    for it in range(NO):
                        nc.tensor.matmul(
                            pc,
                            g_tiles[it][:, jt * P:(jt + 1) * P],
                            t1[it],
                            start=(it == 0),
                            stop=(it == NO - 1),
                        )
                    o2 = stage.tile([P, DOUT], F32, tag="o2")
                    nc.scalar.activation(o2, pc, RELU)
                    nc.sync.dma_start(out_r[:, jt, DOUT:2 * DOUT], o2)

    nc.compile()
    return nc


_NC = None


def _get_nc():
    global _NC
    if _NC is None:
        _NC = build()
    return _NC


def run(inputs: dict, trace: bool = False):
    """Run on 8 cores; returns (stacked_out [B,N,W3], BassKernelResults)."""
    H, G, W = inputs["H"], inputs["G"], inputs["W"]
    H = np.ascontiguousarray(H, dtype=np.float32)
    G = np.ascontiguousarray(G, dtype=np.float32)
    W = np.ascontiguousarray(W, dtype=np.float32)
    in_maps = [
        {"G": np.ascontiguousarray(G[b]), "H": np.ascontiguousarray(H[b]), "W": W}
        for b in range(B)
    ]
    nc = _get_nc()
    res = run_bass_kernel_spmd(nc, in_maps, core_ids=list(range(B)), trace=trace)
    out = np.stack([res.results[b]["out"] for b in range(B)], axis=0)
    return out, res


def kernel(H, G, W):
    out, _ = run({"H": H, "G": G, "W": W})
    return out



# revision 9
# speedup vs baseline: 1.0873x; 1.0873x over previous
"""DirectedGraphConvolution Trainium2 kernel (bf16 restructure).

Per batch element b (one per NeuronCore, 8 total, data-parallel):
    Ne  = H @ W                          [n, dout]
    T1  = G @ Ne   (+ rs = G @ 1)        stream phase, per arriving G tile
    A   : [cs | T2] = G.T @ [1 1 | Ne]   post-stream, fused with
    out2 = relu(G.T @ (T1 / rs))         ... same stationary blocks
    out1 = relu(0.5*(T1 + T2))
    out3 = relu(G @ (T2 / cs))           last sweep, stationary = gt blocks

Schedule: G streams from HBM split across BOTH HWDGE queues (sync: even
tiles, scalar: odd tiles; H/W first so Ne is ready early).  Arriving f32
tiles are cast to bf16 (ACT for sync tiles, DVE for scalar tiles -- the
caster must not sit on the queue-issuing engine or the staging-slot
rotation deadlocks).  Per tile the PE transposes its 16 blocks (bf16
1 cyc/row, bf16 PSUM passthrough -> cheap casts) building a persistent
G^T copy, then runs the 16-matmul T1 accumulation -- so the DMA window
is filled with work that only needs *early* tiles, and everything that
needs *all* of G (pass A) runs post-stream at full PE rate.  rs/cs fall
out of ones-columns fused into the bf16 matmul rhs (no DVE reductions).
Both G (natural, for G.T-left products) and G^T (for G-left) fit
SBUF-resident in bf16.  All matmul streams are >=256 wide (1 cyc/row).
"""

import numpy as np
import concourse.bass as bass
import concourse.mybir as mybir
import concourse.tile as tile
from concourse import bacc
from concourse.bass_utils import run_bass_kernel_spmd
from concourse.masks import make_identity

F32 = mybir.dt.float32
F32R = mybir.dt.float32r
BF16 = mybir.dt.bfloat16
RELU = mybir.ActivationFunctionType.Relu
MULT = mybir.AluOpType.mult
ADD = mybir.AluOpType.add

P = 128
B = 8
N = 2048
NO = N // P            # 16 row tiles
DIN = 256
DOUT = 256
KO = DIN // P          # 2 k tiles for H @ W
W3 = 3 * DOUT
RB = 2 + DOUT          # rhs columns: [1 1 | Ne]
HH = 1024              # half a G tile's columns


def build():
    nc = bacc.Bacc("TRN2", target_bir_lowering=False)
    G = nc.declare_dram_parameter("G", [N, N], F32, isOutput=False)
    H = nc.declare_dram_parameter("H", [N, DIN], F32, isOutput=False)
    W = nc.declare_dram_parameter("W", [DIN, DOUT], F32, isOutput=False)
    out = nc.declare_dram_parameter("out", [N, W3], F32, isOutput=True)

    G_r = G.rearrange("(o p) j -> p o j", p=P)
    H_r = H.rearrange("(o p) d -> p o d", p=P)
    W_r = W.rearrange("(o p) d -> p o d", p=P)
    out_r = out.rearrange("(o p) d -> p o d", p=P)

    with tile.TileContext(nc) as tc:
        with (
            tc.tile_pool(name="const", bufs=1) as const,
            tc.tile_pool(name="gpool", bufs=1) as gpool,
            tc.tile_pool(name="gtpool", bufs=1) as gtpool,
            tc.tile_pool(name="bufp", bufs=1) as bufp,
            tc.tile_pool(name="tpp", bufs=1) as tpp,
            tc.tile_pool(name="stg", bufs=2) as stg,
            tc.tile_pool(name="stage", bufs=1) as stage,
            tc.tile_pool(name="tmpp", bufs=2) as tmpp,
        ):
            # ---------- DMA issue: H/W first, then G interleaved ----------
            hs1 = stg.tile([P, NO // 2, DIN], F32, tag="hs1", bufs=1, name="hs1")
            nc.sync.dma_start(hs1, H_r[:, 0:NO // 2, :])
            ws = const.tile([P, KO, DOUT], F32)
            nc.scalar.dma_start(ws, W_r)
            hs2 = stg.tile([P, NO // 2, DIN], F32, tag="hs2", bufs=1, name="hs2")
            nc.scalar.dma_start(hs2, H_r[:, NO // 2:NO, :])
            # G tiles staged in f32 halves; even tiles on the sync queue,
            # odd on scalar.  Slot-release waits pace each queue; the
            # f32->bf16 casts that free slots run on the OTHER engine.
            gst = {}
            for it in range(NO):
                eng = nc.sync if it % 2 == 0 else nc.scalar
                qt = "gs" if it % 2 == 0 else "gc"
                ha = stg.tile([P, HH], F32, tag=qt + "a", name=f"g{it}a")
                eng.dma_start(ha, G_r[:, it, 0:HH])
                hb = stg.tile([P, HH], F32, tag=qt + "b", bufs=1, name=f"g{it}b")
                eng.dma_start(hb, G_r[:, it, HH:N])
                gst[it] = (ha, hb)

            # ---------- constants / persistent tiles ----------
            ident_f32 = const.tile([P, P], F32)
            make_identity(nc, ident_f32)
            ident16 = const.tile([P, P], BF16)
            nc.vector.tensor_copy(ident16, ident_f32)
            w16 = const.tile([P, KO, DOUT], BF16)
            nc.vector.tensor_copy(w16, ws)
            rs_sb = const.tile([P, NO, 1], F32)

            g16 = [gpool.tile([P, N], BF16, tag=f"g{o}", name=f"g16_{o}")
                   for o in range(NO)]
            gt = [gtpool.tile([P, N], BF16, tag=f"t{o}", name=f"gt_{o}")
                  for o in range(NO)]
            # rhs buffer per block: [1 1 | Ne]  (bf16), packed in one tile
            # so the 516B rows don't each pad to a full slot
            bufall = bufp.tile([P, NO, RB], BF16, name="bufall")
            buf = [bufall[:, o, :] for o in range(NO)]
            t1p = [tpp.tile([P, DOUT], BF16, tag=f"p{o}", name=f"t1p{o}")
                   for o in range(NO)]
            t2p = [tpp.tile([P, DOUT], BF16, tag=f"q{o}", name=f"t2p{o}")
                   for o in range(NO)]
            for o in range(NO):
                nc.gpsimd.memset(buf[o][:, 0:2], 1.0)

            # ---------- Ne = H @ W ----------
            # H blocks transposed on PE straight from the f32 staging
            # (f32r, 1.5 cyc/row), cast to bf16, matmul'd against W.
            with (
                tc.tile_pool(name="ps_ht", bufs=2, space="PSUM") as ps_ht,
                tc.tile_pool(name="ps_ne", bufs=2, space="PSUM") as ps_ne,
            ):
                for rnd in range(2):
                    hs = hs1 if rnd == 0 else hs2
                    hts = stg.tile([P, 8 * KO * P], BF16, tag="hts", bufs=1,
                                   name=f"hts{rnd}")
                    for q in range(4):  # 4 psum fills of 4 transposes
                        pq = ps_ht.tile([P, 4 * P], F32, tag="pht")
                        for u in range(4):
                            blk = q * 4 + u          # t-kt block index in hs
                            t, kt = blk // KO, blk % KO
                            nc.tensor.transpose(
                                pq[:, u * P:(u + 1) * P],
                                hs[:, t, kt * P:(kt + 1) * P],
                                ident_f32,
                            )
                        eng = nc.vector if q % 2 == 0 else nc.scalar
                        if q % 2 == 0:
                            nc.vector.tensor_copy(
                                hts[:, q * 4 * P:(q + 1) * 4 * P], pq)
                        else:
                            nc.scalar.copy(
                                hts[:, q * 4 * P:(q + 1) * 4 * P], pq)
                    for t in range(8):
                        tglob = rnd * 8 + t
                        pne = ps_ne.tile([P, DOUT], F32, tag="pne")
                        for kt in range(KO):
                            nc.tensor.matmul(
                                pne,
                                hts[:, (t * KO + kt) * P:(t * KO + kt + 1) * P],
                                w16[:, kt, :],
                                start=(kt == 0),
                                stop=(kt == KO - 1),
                            )
                        if t % 2 == 0:
                            nc.vector.tensor_copy(buf[tglob][:, 2:RB], pne)
                        else:
                            nc.scalar.copy(buf[tglob][:, 2:RB], pne)

            # ---------- stream phase: per G tile ----------
            # cast f32->bf16, transpose 16 blocks (bf16 PSUM), cast to gt,
            # then T1[it] (+rs) = gt[it].T-blocks @ [1 1 | Ne].
            with (
                tc.tile_pool(name="ps_tr", bufs=2, space="PSUM") as ps_tr,
                tc.tile_pool(name="ps_t1", bufs=2, space="PSUM") as ps_t1,
            ):
                for it in range(NO):
                    ha, hb = gst[it]
                    cast_g = nc.scalar if it % 2 == 0 else nc.vector
                    cast_t = nc.vector if it % 2 == 0 else nc.scalar
                    if it % 2 == 0:
                        cast_g.copy(g16[it][:, 0:HH], ha)
                        cast_g.copy(g16[it][:, HH:N], hb)
                    else:
                        cast_g.tensor_copy(g16[it][:, 0:HH], ha)
                        cast_g.tensor_copy(g16[it][:, HH:N], hb)
                    tra = ps_tr.tile([P, 8 * P], BF16, tag="tra")
                    trb = ps_tr.tile([P, 8 * P], BF16, tag="trb")
                    for jt in range(8):
                        nc.tensor.transpose(
                            tra[:, jt * P:(jt + 1) * P],
                            g16[it][:, jt * P:(jt + 1) * P],
                            ident16,
                        )
                    for jt in range(8, NO):
                        nc.tensor.transpose(
                            trb[:, (jt - 8) * P:(jt - 7) * P],
                            g16[it][:, jt * P:(jt + 1) * P],
                            ident16,
                        )
                    if it % 2 == 0:
                        cast_t.tensor_copy(gt[it][:, 0:HH], tra)
                        cast_t.tensor_copy(gt[it][:, HH:N], trb)
                    else:
                        cast_t.copy(gt[it][:, 0:HH], tra)
                        cast_t.copy(gt[it][:, HH:N], trb)
                    pt1 = ps_t1.tile([P, RB], F32, tag="pt1")
                    for jt in range(NO):
                        nc.tensor.matmul(
                            pt1,
                            gt[it][:, jt * P:(jt + 1) * P],
                            buf[jt][:, 0:RB],
                            start=(jt == 0),
                            stop=(jt == NO - 1),
                        )
                    # epilogue: rs, T1' = T1/rs (bf16)
                    nc.vector.tensor_copy(rs_sb[:, it, :], pt1[:, 0:1])
                    rsinv = tmpp.tile([P, 1], F32, tag="rsi")
                    nc.vector.reciprocal(rsinv, pt1[:, 0:1])
                    nc.vector.tensor_scalar_mul(
                        t1p[it], pt1[:, 2:RB], rsinv[:, 0:1])

            # ---------- fused pass A + out2 (stationary = g16 blocks) ----
            # pa = G.T @ [1 1 | Ne] -> [cs | T2];  po2 = G.T @ T1'
            with (
                tc.tile_pool(name="ps_a", bufs=2, space="PSUM") as ps_a,
                tc.tile_pool(name="ps_o2", bufs=2, space="PSUM") as ps_o2,
            ):
                for jt in range(NO):
                    pa = ps_a.tile([P, RB], F32, tag="pa")
                    po2 = ps_o2.tile([P, DOUT], F32, tag="po2")
                    for it in range(NO):
                        nc.tensor.matmul(
                            pa,
                            g16[it][:, jt * P:(jt + 1) * P],
                            buf[it][:, 0:RB],
                            start=(it == 0),
                            stop=(it == NO - 1),
                        )
                        nc.tensor.matmul(
                            po2,
                            g16[it][:, jt * P:(jt + 1) * P],
                            t1p[it],
                            start=(it == 0),
                            stop=(it == NO - 1),
                        )
                    # epilogue: T2' = T2/cs; out1 = relu(0.5(T1 + T2));
                    # out2 = relu(po2)
                    csinv = tmpp.tile([P, 1], F32, tag="csi")
                    nc.vector.reciprocal(csinv, pa[:, 0:1])
                    nc.vector.tensor_scalar_mul(
                        t2p[jt], pa[:, 2:RB], csinv[:, 0:1])
                    o1p = tmpp.tile([P, DOUT], F32, tag="o1p", bufs=1)
                    nc.vector.scalar_tensor_tensor(
                        o1p, t1p[jt], rs_sb[:, jt, :], pa[:, 2:RB], MULT, ADD)
                    o1 = stage.tile([P, DOUT], F32, tag="o", bufs=2, name="o1")
                    nc.scalar.activation(o1, o1p, RELU, scale=0.5)
                    nc.sync.dma_start(out_r[:, jt, 0:DOUT], o1)
                    o2 = stage.tile([P, DOUT], F32, tag="o", bufs=2, name="o2")
                    nc.scalar.activation(o2, po2, RELU)
                    nc.sync.dma_start(out_r[:, jt, DOUT:2 * DOUT], o2)

            # ---------- out3 = relu(G @ T2') (stationary = gt blocks) ----
            with tc.tile_pool(name="ps_o3", bufs=3, space="PSUM") as ps_o3:
                for it in range(NO):
                    po3 = ps_o3.tile([P, DOUT], F32, tag="po3")
                    for jt in range(NO):
                        nc.tensor.matmul(
                            po3,
                            gt[it][:, jt * P:(jt + 1) * P],
                            t2p[jt],
                            start=(jt == 0),
                            stop=(jt == NO - 1),
                        )
                    o3 = stage.tile([P, DOUT], F32, tag="o", bufs=2, name="o3")
                    nc.scalar.activation(o3, po3, RELU)
                    nc.sync.dma_start(out_r[:, it, 2 * DOUT:W3], o3)

    nc.compile()
    return nc


_NC = None


def _get_nc():
    global _NC
    if _NC is None:
        _NC = build()
    return _NC


def run(inputs: dict, trace: bool = False):
    """Run on 8 cores; returns (stacked_out [B,N,W3], BassKernelResults)."""
    H, G, W = inputs["H"], inputs["G"], inputs["W"]
    H = np.ascontiguousarray(H, dtype=np.float32)
    G = np.ascontiguousarray(G, dtype=np.float32)
    W = np.ascontiguousarray(W, dtype=np.float32)
    in_maps = [
        {"G": np.ascontiguousarray(G[b]), "H": np.ascontiguousarray(H[b]), "W": W}
        for b in range(B)
    ]
    nc = _get_nc()
    res = run_bass_kernel_spmd(nc, in_maps, core_ids=list(range(B)), trace=trace)
    out = np.stack([res.results[b]["out"] for b in range(B)], axis=0)
    return out, res


def kernel(H, G, W):
    out, _ = run({"H": H, "G": G, "W": W})
    return out
